# revision 43
# baseline (speedup 1.0000x reference)
"""MiniBatchDiscrimination Trainium2 kernel (8-core SPMD).

Reference computation:
    m = (x @ T).reshape(B, OUT_F, NUM_K)            # B=256, OUT_F=128, NUM_K=16
    dists = |m[None,:,:,:] - m[:,None,:,:]|         # [B, B, OUT_F, NUM_K]
    out = sum_i exp(-sum_k dists) - 1               # [B, OUT_F]
    return concat([x, out], axis=-1)                # [B, 640]

Strategy (per core, identical SPMD program, per-core data):
  * Each core owns JB=32 output rows (j). Full m is computed on every core
    (replicated GEMM, cheap) so no collectives are needed.
  * m is stored in SBUF as [partition p=(f8,k), free n=(i,f_o)] with
    f = f_o*8 + f8, p = f8*16 + k. The GEMM runs in fp8 DoubleRow mode
    (contraction packed in pairs, 2 matmuls per f_o).
  * Pairwise pass: uniform hybrid blocks of 16 i's. DVE bf16 tensor_sub
    (2x mode) makes the diff; i's [0, ACT_I) go ACT Abs -> fp8 -> one
    DoubleRow matmul per i-PAIR (the 256-deep virtual contraction k-sums
    both i's at 2 cols/cycle); i's [ACT_I, 16) go DVE 4x sign-strip abs ->
    block-diagonal bf16 ones matmul per i. All matmuls accumulate into one
    PSUM bank holding dist rows [128=(i_sub,f8), 512=(j,f_o)].
  * exp(-dist): ACT Exp with scale=-1 from PSUM -> fp8 SBUF, block pairs
    share an [128, 2, 512] tile; i-sum via fp8 DoubleRow accumulation.
  * Host unshards: reshape to [32,128] per core, concat with x.
"""

import os
import numpy as np

import concourse.bass as bass
import concourse.tile as tile
from concourse import bacc, mybir

BF16 = mybir.dt.bfloat16
FP32 = mybir.dt.float32
FP8 = mybir.dt.float8e4
NPBF16 = np.dtype(mybir.dt.np(BF16))
NPFP8 = np.dtype(mybir.dt.np(FP8))

B = 256
IN_F = 512
OUT_F = 128
NUM_K = 16
N_CORES = 8
JB = B // N_CORES          # 32 j-rows owned per core
F8 = 8                     # f8 = f % 8   (partition group)
FO = OUT_F // F8           # 16 f_o values (free dim)
NBLK = B // 16             # 16 i-blocks of 16
# Per 16-i block: i's [0, ACT_I) take the fp8 path (ACT abs -> fp8 ->
# DoubleRow matmul per i-pair), i's [ACT_I, 16) take the bf16 path (DVE 4x
# abs -> ones_k matmul per i). Uniform hybrid blocks keep all three engines
# fed at every point of the main loop. ACT_I must be even.
ACT_I = int(os.environ.get("ACT_I", "10"))


def build_nc():
    nc = bacc.Bacc(name="minibatch_discrim")

    # host-prearranged fp8, contraction packed in pairs for DoubleRow:
    # xT8[p, ch, t, i] = x[i, ch*256 + t*128 + p]; columns B..B+JB repeat this
    # core's own j-columns so one FD=288 matmul produces m_all and m_sh
    # together (identical values -> exact diagonal).
    xT_d = nc.dram_tensor("xT", [128, 2, 2, B + JB], FP8, kind="ExternalInput")
    # T8[p, c4, fo4, ch, t, n] = T[ch*256 + t*128 + p, (c4*4+fo4)*128 + n] —
    # fo inside the partition line (2KB per 4-fo chunk), 4 chunked DMAs.
    T_d = nc.dram_tensor("T_w", [128, 4, 4, 2, 2, 128], FP8, kind="ExternalInput")
    onk_d = nc.dram_tensor("ones_k", [128, 8 * 64], BF16, kind="ExternalInput")
    # DoubleRow stationary: ones_dr[p, q, t, r] = 1 iff r == q*16 + t*8 + p//16
    # (q = i-pair index 0..7, r over all 128 rows). One fp8 matmul k-reduces
    # an i-PAIR into rows 16q + t*8 + f8; 8 matmuls accumulate to fill the
    # [128, 512] dist bank (DoubleRow dst must start at partition 0).
    ondr_d = nc.dram_tensor("ones_dr", [128, 8 * 2 * 128], FP8, kind="ExternalInput")
    # DoubleRow acc stationary, padded to 32 rows (<=64-col DR matmuls stream
    # 2x faster): ones_adr[p, t, r] = 1 iff r == p % 8; rows 8..31 stay zero.
    onadr_d = nc.dram_tensor("ones_adr", [128, 2 * 32], FP8, kind="ExternalInput")
    out_d = nc.dram_tensor("out_pair", [F8, JB * FO], FP32, kind="ExternalOutput")

    with tile.TileContext(nc) as tc:
        with (
            tc.tile_pool(name="const", bufs=1) as constp,
            tc.tile_pool(name="mm", bufs=1) as mmp,
            tc.tile_pool(name="gpsum", bufs=4, space=bass.MemorySpace.PSUM) as gps,
            tc.tile_pool(name="dpsum", bufs=3, space=bass.MemorySpace.PSUM) as dps,
            tc.tile_pool(name="apsum", bufs=1, space=bass.MemorySpace.PSUM) as aps,
            tc.tile_pool(name="work", bufs=3) as wp,
            tc.tile_pool(name="work8", bufs=3) as w8p,
            tc.tile_pool(name="expp", bufs=3) as ep,
        ):
            # ---- constants / inputs to SBUF ----
            zero_b = constp.tile([128, 1], FP32)
            nc.gpsimd.memset(zero_b[:], 0.0)

            # ones_k[:, q8, (q, f8)] = 1 iff q == q8 and p//16 == f8.
            # The k-reduce matmul for i_sub targets the 64-partition slice at
            # offset (isub//8)*64 using pattern q8 = isub%8: its 8 target rows
            # get sum_k, the other 56 rows of the slice accumulate += 0.
            ones_k = constp.tile([128, 8, 64], BF16)
            nc.sync.dma_start(ones_k[:], onk_d.rearrange("p (s q) -> p s q", q=64))
            ones_dr = constp.tile([128, 8, 2, 128], FP8)
            nc.sync.dma_start(
                ones_dr[:], ondr_d.rearrange("p (q t r) -> p q t r", t=2, r=128)
            )
            ones_adr = constp.tile([128, 2, 32], FP8)
            nc.sync.dma_start(ones_adr[:], onadr_d.rearrange("p (t r) -> p t r", r=32))

            # warm the ACT exp/abs table while DMAs run
            warm = constp.tile([128, 1], FP32)
            nc.scalar.activation(
                warm[:], zero_b[:], mybir.ActivationFunctionType.Exp, bias=zero_b[:]
            )

            # xT as [p, ch, t, i]  (fp8, DoubleRow-packed contraction)
            xT_sb = constp.tile([128, 2, 2, B + JB], FP8)
            nc.sync.dma_start(xT_sb[:], xT_d[:])
            # T in 4 chunked DMAs (2KB per-partition lines) so the first
            # fo-GEMMs start as soon as their chunk lands
            T_sb = constp.tile([128, FO, 2, 2, 128], FP8)
            for c4 in range(4):
                nc.sync.dma_start(T_sb[:, c4 * 4:(c4 + 1) * 4], T_d[:, c4])

            # ---- GEMM: m_ext [p=(f8,k), (i or dup-j, f_o)] ----
            # 2 DoubleRow matmuls per fo (256-deep virtual contraction each).
            # The strided PSUM->SBUF rearrange into [i, fo] is the prologue
            # long pole (~0.9-1.6us per fo on any one engine), so spread it:
            # DVE takes 8 fo's directly from PSUM, ACT 4, GpSimd 4 (via a
            # contiguous DVE downcast, since GpSimd can't read PSUM). GpSimd
            # SBUF contention is harmless here — DVE is idle pre-subs.
            m_stage = mmp.tile([128, 4, B + JB], BF16)
            m_ext = mmp.tile([128, B + JB, FO], BF16)
            for fo in range(FO):
                pm = gps.tile([128, B + JB], FP32, tag="gemm_full")
                for ch in range(2):
                    nc.tensor.matmul(
                        pm[:],
                        T_sb[:, fo, ch],
                        xT_sb[:, ch],
                        start=(ch == 0),
                        stop=(ch == 1),
                        perf_mode=mybir.MatmulPerfMode.DoubleRow,
                    )
                if fo % 4 < 2:
                    nc.vector.tensor_copy(m_ext[:, :, fo], pm[:])
                elif fo % 4 == 2:
                    nc.scalar.copy(m_ext[:, :, fo], pm[:])
                else:
                    nc.vector.tensor_copy(m_stage[:, fo // 4, :], pm[:])
                    nc.gpsimd.tensor_copy(
                        m_ext[:, :, fo], m_stage[:, fo // 4, :]
                    )

            # ---- main pairwise loop (uniform hybrid blocks) ----
            # sum over i of exp(-dist); rows 8..31 are zero padding (the acc
            # DoubleRow stationary is padded to 32 rows)
            acc = aps.tile([32, JB * FO], FP32)
            for blk in range(NBLK):
                pd = dps.tile([128, JB * FO], FP32, tag="dist")
                diff = wp.tile([128, 16, JB, FO], BF16, tag="diff")
                nc.vector.tensor_sub(
                    diff[:],
                    m_ext[:, None, B:, :].broadcast_to([128, 16, JB, FO]),
                    m_ext[:, blk * 16:(blk + 1) * 16, None, :].broadcast_to(
                        [128, 16, JB, FO]
                    ),
                )
                if ACT_I > 0:
                    # fp8 share: ACT does abs + fp8 convert in one pass;
                    # one DoubleRow matmul k-reduces each i-PAIR.
                    ad8 = w8p.tile([128, ACT_I, JB, FO], FP8, tag="absdr")
                    nc.scalar.activation(
                        ad8[:], diff[:, :ACT_I],
                        mybir.ActivationFunctionType.Abs, bias=zero_b[:],
                    )
                    for q in range(ACT_I // 2):
                        nc.tensor.matmul(
                            pd[:],
                            ones_dr[:, q],
                            ad8[:, 2 * q:2 * q + 2, :, :],
                            start=(q == 0),
                            stop=(ACT_I == 16 and q == 7),
                            perf_mode=mybir.MatmulPerfMode.DoubleRow,
                            skip_group_check=True,
                        )
                if ACT_I < 16:
                    # bf16 share: |x| on DVE at 4x mode (strip sign bit),
                    # block-diagonal ones matmul per i, accumulating into the
                    # same PSUM bank as the DoubleRow share.
                    ad = wp.tile([128, 16 - ACT_I, JB, FO], BF16, tag="absd")
                    nc.vector.tensor_scalar(
                        ad[:].bitcast(mybir.dt.uint16),
                        diff[:, ACT_I:].bitcast(mybir.dt.uint16),
                        0x7FFF, None, op0=mybir.AluOpType.bitwise_and,
                    )
                    for s in range(16 - ACT_I):
                        isub = ACT_I + s
                        g, q = isub // 8, isub % 8
                        nc.tensor.matmul(
                            pd[g * 64:(g + 1) * 64, :],
                            ones_k[:, q, :],
                            ad[:, s, :, :],
                            start=(ACT_I == 0 and isub == 0),
                            stop=(isub == 15),
                            skip_group_check=True,
                        )
                if blk % 2 == 0:
                    et2 = ep.tile([128, 2, JB * FO], FP8, tag="expt")
                nc.scalar.activation(
                    et2[:, blk % 2, :], pd[:],
                    mybir.ActivationFunctionType.Exp, bias=zero_b[:], scale=-1.0,
                )
                if blk % 2 == 1:
                    # i-sum of two exp-blocks in one fp8 DoubleRow matmul
                    nc.tensor.matmul(
                        acc[:],
                        ones_adr[:],
                        et2[:],
                        start=(blk == 1),
                        stop=(blk == NBLK - 1),
                        perf_mode=mybir.MatmulPerfMode.DoubleRow,
                        skip_group_check=True,
                    )

            # ---- tail: subtract 1, store ----
            fin = mmp.tile([F8, JB * FO], FP32)
            nc.vector.tensor_scalar_add(fin[:], acc[:F8, :], -1.0)
            nc.sync.dma_start(out_d[:], fin[:])

    nc.finalize()
    return nc


def make_in_maps(x: np.ndarray, T: np.ndarray):
    # xT8[p, ch, t, i] = x[i, ch*256 + t*128 + p]  (fp8 DoubleRow packing)
    xT_h = np.ascontiguousarray(
        x.T.astype(NPFP8).reshape(2, 2, 128, B).transpose(2, 0, 1, 3)
    )
    # T8[p, fo, ch, t, n] = T[ch*256 + t*128 + p, fo*128 + n]
    T_perm = np.ascontiguousarray(
        T.astype(NPFP8).reshape(2, 2, 128, FO, 128).transpose(2, 3, 0, 1, 4)
    )

    p = np.arange(128)[:, None]
    # ones_k[p, q8, q] = 1 iff q == q8*8 + p//16  (q in 0..63)
    q = np.arange(64)[None, None, :]
    s = np.arange(8)[None, :, None]
    ones_k = (q == s * 8 + p[:, :, None] // 16).astype(NPBF16)
    ones_k = np.ascontiguousarray(ones_k.reshape(128, 8 * 64))
    # ones_dr[p, (q, t, r)] = 1 iff r == q*16 + t*8 + p//16  (r in 0..127)
    qq = np.arange(8)[None, :, None, None]
    t = np.arange(2)[None, None, :, None]
    r128 = np.arange(128)[None, None, None, :]
    ones_dr = np.ascontiguousarray(
        (r128 == qq * 16 + t * 8 + p[:, :, None, None] // 16)
        .astype(NPFP8).reshape(128, 8 * 2 * 128)
    )
    # ones_adr[p, t*32 + r] = 1 iff r == p % 8  (rows 8..31 zero padding)
    r32 = np.arange(32)[None, None, :]
    ones_adr = np.ascontiguousarray(
        np.broadcast_to(r32 == p[:, :, None] % 8, (128, 2, 32))
        .astype(NPFP8).reshape(128, 64)
    )

    in_maps = []
    for c in range(N_CORES):
        xTc = np.ascontiguousarray(np.concatenate(
            [xT_h, xT_h[:, :, :, c * JB:(c + 1) * JB]], axis=3
        ))
        in_maps.append({
            "xT": xTc,
            "T_w": T_perm,
            "ones_k": ones_k,
            "ones_dr": ones_dr,
            "ones_adr": ones_adr,
        })
    return in_maps


def assemble(x: np.ndarray, pair_parts) -> np.ndarray:
    """pair_parts: list of [8, JB*FO] fp32 per core -> full [B, IN_F+OUT_F]."""
    out = np.empty((B, IN_F + OUT_F), np.float32)
    out[:, :IN_F] = x
    for c, fp in enumerate(pair_parts):
        # fp[f8, j*FO + fo] -> out[c*JB + j, IN_F + fo*8 + f8]
        blk = fp.reshape(F8, JB, FO).transpose(1, 2, 0).reshape(JB, OUT_F)
        out[c * JB:(c + 1) * JB, IN_F:] = blk
    return out


_NC_CACHE = None


def kernel(x: np.ndarray, T: np.ndarray) -> np.ndarray:
    global _NC_CACHE
    from concourse import bass_utils

    if _NC_CACHE is None:
        _NC_CACHE = build_nc()
    nc = _NC_CACHE
    in_maps = make_in_maps(np.asarray(x, np.float32), np.asarray(T, np.float32))
    res = bass_utils.run_bass_kernel_spmd(nc, in_maps, core_ids=list(range(N_CORES)))
    parts = [r["out_pair"].astype(np.float32) for r in res.results]
    return assemble(np.asarray(x, np.float32), parts)

